# revision 49
# baseline (speedup 1.0000x reference)
"""Trainium2 Bass kernel for nn_DGL_Net (3-layer GraphConv GNN, 50000 nodes, 800k edges).

Strategy (8 NeuronCores, SPMD):
  - Host: relabel nodes into 392 balanced tiles of 128 nodes (<=2046 in-edges per
    tile), 49 tiles per core. Per layer: local matmul (bf16) -> scale by c_src ->
    AllGather of the per-node activations -> per-edge dma_gather (4 SWDGE queues,
    12-deep pipelining so the drain runs at steady state) -> one-hot (Sel) matmul
    aggregation in PSUM -> scale by c_dst + bias (+relu / log_softmax).
  - AllGather 1 is split into 4 sub-collectives over a block-interleaved global
    row layout so the transfers overlap the tail of the M1 matmuls (L2/L3 use a
    separate core-major layout with their own idx table).
  - Sel one-hots are built on-chip (DVE is_equal of per-slot dst lanes against an
    iota), one [128, 1024] op per gather call; no selc DMA. Padding slots carry
    dst_local=-1 so their Sel column is all-zero.
  - Layer 3 uses Y3 = (A @ H2c) @ W3 (associativity): H2c (32-wide) is
    AllGathered/gathered instead of M3, and W3 is applied per tile after the
    aggregation; there is no separate M3 phase. log_softmax is batched per 8
    tiles to avoid per-tile cross-engine ping-pong.
  - PSUM drains run on the scalar engine (activation Copy/Relu with per-partition
    scale) -- small-free-dim PSUM reads on the DVE are pathologically slow.
  - int16 gather indices: gather base is offset +32768 rows so idx = row-32768
    spans the whole [0, 50176) row space within int16. The last slot of every
    1024-index gather call is a reserved dummy with idx>=0 (defeats the ucode's
    trailing-negative trim). num_idxs per dma_gather call is capped at 1024 by
    the ucode.
"""
import os
import sys

sys.path.insert(0, '/opt/trn_rl_repo')

import numpy as np
import ml_dtypes

import concourse.bass as bass
import concourse.bacc as bacc
import concourse.mybir as mybir
import concourse.tile as tile
from concourse.bass_utils import run_bass_kernel_spmd

BF16 = ml_dtypes.bfloat16

N_NODES = 50000
N_CORES = 8
TILE_N = 128                 # nodes per tile
TILES_PER_CORE = 49
AG_TB = [13, 12, 12, 12]     # tiles per AllGather sub-block (sum = 49)
AG_BLOCKS = len(AG_TB)
N_TILES = N_CORES * TILES_PER_CORE      # 392
ROWS_PER_CORE = TILES_PER_CORE * TILE_N  # 6272
N_ROWS = N_CORES * ROWS_PER_CORE         # 50176
R_CHUNKS = 16                # edge chunks (of 128 slots) per tile
SLOTS_PER_TILE = R_CHUNKS * 128          # 2048
TILE_EDGE_CAP = SLOTS_PER_TILE - 2       # 2046 (2 reserved call-end dummies)
SLOTS = TILES_PER_CORE * SLOTS_PER_TILE  # 100352 per core
CALL = 1024                  # idxs per dma_gather call
N_CALLS = SLOTS // CALL      # 98
CHUNKS = TILES_PER_CORE * R_CHUNKS       # 784 chunks per core
IDX_OFF = 32768              # gather base offset (int16 trick)
F_IN = 1433
F_IN_P = 1536                # padded to 12*128
KC1 = F_IN_P // 128          # 12
F1 = 256
F2 = 32
F3 = 7
FPAD = 128                   # padded row width for M2/M3 gather (256B elems)

last_exec_time_ns = None


def _preprocess(edge_index):
    """Graph preprocessing: normalization constants, node->($core,tile,lane)
    relabeling with balanced per-tile in-degree, per-core edge slot tables."""
    src = np.asarray(edge_index[0], dtype=np.int64)
    dst = np.asarray(edge_index[1], dtype=np.int64)
    n_edges = src.shape[0]

    deg_out = np.bincount(src, minlength=N_NODES).astype(np.float64)
    deg_in = np.bincount(dst, minlength=N_NODES).astype(np.float64)
    c_src = (1.0 / np.sqrt(np.maximum(deg_out, 1.0))).astype(np.float32)
    c_dst = (1.0 / np.sqrt(np.maximum(deg_in, 1.0))).astype(np.float32)

    # --- greedy balanced tile packing by in-degree ---
    import heapq
    order = np.argsort(-deg_in, kind='stable')
    heap = [(0.0, 0, t) for t in range(N_TILES)]  # (load, count, tile)
    heapq.heapify(heap)
    tile_nodes = [[] for _ in range(N_TILES)]
    tile_load = np.zeros(N_TILES)
    deferred = []
    for v in order:
        dv = deg_in[v]
        while True:
            load, cnt, t = heapq.heappop(heap)
            if cnt >= TILE_N:
                continue  # stale/full
            if load + dv > TILE_EDGE_CAP:
                deferred.append((load, cnt, t))
                continue
            break
        tile_nodes[t].append(int(v))
        tile_load[t] = load + dv
        heapq.heappush(heap, (load + dv, cnt + 1, t))
        for item in deferred:
            heapq.heappush(heap, item)
        deferred = []
    assert max(tile_load) <= TILE_EDGE_CAP

    # sort tiles by load desc, group by 8, core c takes c-th of each group
    tsort = np.argsort(-tile_load, kind='stable')
    # tile_of_slot[core][k] = global tile id
    tile_assign = np.empty((N_CORES, TILES_PER_CORE), dtype=np.int64)
    for k in range(TILES_PER_CORE):
        for c in range(N_CORES):
            tile_assign[c, k] = tsort[k * N_CORES + c]

    # local row mapping: lrow = c*ROWS_PER_CORE + k*128 + lane  (per-core I/O order)
    # global row mapping (AllGather result layout, block-interleaved so the
    # AllGather can be split into AG_BLOCKS sub-collectives):
    #   grow = 8*128*sum(AG_TB[:b]) + c*AG_TB[b]*128 + (k-k0[b])*128 + lane
    node_of_row = np.full(N_ROWS, -1, dtype=np.int64)  # -1 = virtual pad node
    grow_of_node = np.full(N_NODES, -1, dtype=np.int64)
    node_c = np.full(N_NODES, -1, dtype=np.int64)
    node_k = np.full(N_NODES, -1, dtype=np.int64)
    node_lane = np.full(N_NODES, -1, dtype=np.int64)
    blk_of_k = np.concatenate([np.full(t, b) for b, t in enumerate(AG_TB)])
    k0_of_blk = np.concatenate([[0], np.cumsum(AG_TB)])[:-1]
    gbase_of_blk = 8 * TILE_N * np.concatenate([[0], np.cumsum(AG_TB)])[:-1]
    for c in range(N_CORES):
        for k in range(TILES_PER_CORE):
            t = tile_assign[c, k]
            nodes = tile_nodes[t]
            base = c * ROWS_PER_CORE + k * TILE_N
            b = blk_of_k[k]
            gbase = gbase_of_blk[b] + c * AG_TB[b] * TILE_N + (k - k0_of_blk[b]) * TILE_N
            for lane, v in enumerate(nodes):
                node_of_row[base + lane] = v
                grow_of_node[v] = gbase + lane
                node_c[v] = c
                node_k[v] = k
                node_lane[v] = lane
    assert (grow_of_node >= 0).all()

    # --- per-core edge slot tables ---
    # L1 gathers read the block-interleaved (split-AG) layout; L2/L3 read the
    # core-major layout of the single AllGathers. Same slots, two idx tables.
    lrow_of_node = node_c * ROWS_PER_CORE + node_k * TILE_N + node_lane
    src_row1 = grow_of_node[src]
    src_row23 = lrow_of_node[src]
    e_core = node_c[dst]
    e_tile = node_k[dst]              # k within core
    e_lane = node_lane[dst]

    idx1_flat = np.zeros((N_CORES, SLOTS), dtype=np.int16)     # pad idx = 0
    idx23_flat = np.zeros((N_CORES, SLOTS), dtype=np.int16)
    dst_flat = np.full((N_CORES, SLOTS), -1, dtype=np.int16)   # pad dst = -1

    # group edges by (core, tile) and assign slot positions
    key = e_core * TILES_PER_CORE + e_tile
    eorder = np.argsort(key, kind='stable')
    key_s = key[eorder]
    # position within group
    grp_start = np.searchsorted(key_s, np.arange(N_CORES * TILES_PER_CORE))
    pos_in_grp = np.arange(n_edges) - grp_start[key_s]
    # slot within tile: skip reserved slots 1023 and 2047
    j = pos_in_grp
    slot_in_tile = j + (j >= 1023).astype(np.int64) # j>=1023 shifts past slot 1023
    assert slot_in_tile.max() < SLOTS_PER_TILE - 1  # never hits 2047
    slots_abs = key_s % TILES_PER_CORE * SLOTS_PER_TILE + slot_in_tile
    cores_s = key_s // TILES_PER_CORE
    idx1_flat[cores_s, slots_abs] = (src_row1[eorder] - IDX_OFF).astype(np.int16)
    idx23_flat[cores_s, slots_abs] = (src_row23[eorder] - IDX_OFF).astype(np.int16)
    dst_flat[cores_s, slots_abs] = e_lane[eorder].astype(np.int16)

    # wrap idx to [128, SLOTS/16] (idx i -> [i%16 replicated, i//16])
    cols = SLOTS // 16
    idx_tile1 = np.zeros((N_CORES, 128, cols), dtype=np.int16)
    idx_tile23 = np.zeros((N_CORES, 128, cols), dtype=np.int16)
    for c in range(N_CORES):
        idx_tile1[c] = np.tile(idx1_flat[c].reshape(cols, 16).T, (8, 1))
        idx_tile23[c] = np.tile(idx23_flat[c].reshape(cols, 16).T, (8, 1))
    # per-slot dst lane, [128 lane, CHUNKS] int16 (-1 pad); Sel one-hots are
    # built on-chip via is_equal against an iota.
    dst16 = dst_flat.reshape(N_CORES, CHUNKS, 128).transpose(0, 2, 1).copy()

    # per-core normalization tables
    cd_row = np.where(node_of_row >= 0, c_dst[np.maximum(node_of_row, 0)], 1.0)
    cs_row = np.where(node_of_row >= 0, c_src[np.maximum(node_of_row, 0)], 1.0)
    cd_core = cd_row.reshape(N_CORES, ROWS_PER_CORE).astype(np.float32)
    cs_core = cs_row.reshape(N_CORES, ROWS_PER_CORE).astype(np.float32)
    cdst_rep = np.repeat(cd_core[:, None, :], 128, axis=1)  # [C,128,6272]
    cdst_pp = cd_core.reshape(N_CORES, TILES_PER_CORE, 128).transpose(0, 2, 1).copy()
    csrc_t = cs_core.reshape(N_CORES, TILES_PER_CORE, 128).transpose(0, 2, 1).copy()

    return dict(node_of_row=node_of_row,
                idx_tile1=idx_tile1, idx_tile23=idx_tile23, dst16=dst16,
                cdst_rep=cdst_rep.astype(np.float32), cdst_pp=cdst_pp,
                csrc_t=csrc_t)


def _build_nc():
    nc = bacc.Bacc("TRN2", target_bir_lowering=False, debug=False,
                   enable_asserts=True, num_devices=N_CORES, num_swdge_queues=4)
    dt = mybir.dt
    inp = {}
    inp['xT'] = nc.dram_tensor("xT", [F_IN_P, ROWS_PER_CORE], dt.bfloat16, kind="ExternalInput")
    inp['W1'] = nc.dram_tensor("W1", [F_IN_P, F1], dt.bfloat16, kind="ExternalInput")
    inp['W2'] = nc.dram_tensor("W2", [F1, F2], dt.bfloat16, kind="ExternalInput")
    inp['W3'] = nc.dram_tensor("W3", [F2, F3], dt.bfloat16, kind="ExternalInput")
    inp['idx1'] = nc.dram_tensor("idx1", [128, SLOTS // 16], dt.int16, kind="ExternalInput")
    inp['idx23'] = nc.dram_tensor("idx23", [128, SLOTS // 16], dt.int16, kind="ExternalInput")
    inp['dst16'] = nc.dram_tensor("dst16", [128, CHUNKS], dt.int16, kind="ExternalInput")
    inp['cdst_rep'] = nc.dram_tensor("cdst_rep", [128, ROWS_PER_CORE], dt.float32, kind="ExternalInput")
    inp['cdst_pp'] = nc.dram_tensor("cdst_pp", [128, TILES_PER_CORE], dt.float32, kind="ExternalInput")
    inp['csrc_t'] = nc.dram_tensor("csrc_t", [128, TILES_PER_CORE], dt.float32, kind="ExternalInput")
    inp['b1pp'] = nc.dram_tensor("b1pp", [128, 2], dt.float32, kind="ExternalInput")
    inp['b2row'] = nc.dram_tensor("b2row", [128, F2], dt.float32, kind="ExternalInput")
    inp['b3t'] = nc.dram_tensor("b3t", [128, F3], dt.float32, kind="ExternalInput")
    out_t = nc.dram_tensor("out", [ROWS_PER_CORE, F3], dt.float32, kind="ExternalOutput")

    m1_own = [nc.dram_tensor(f"m1_own{b}", [tb * 128, F1], dt.bfloat16)
              for b, tb in enumerate(AG_TB)]
    m1_full = nc.dram_tensor("m1_full", [N_ROWS, F1], dt.bfloat16, addr_space="Shared")
    m2_own = nc.dram_tensor("m2_own", [ROWS_PER_CORE, FPAD], dt.bfloat16)
    m2_full = nc.dram_tensor("m2_full", [N_ROWS, FPAD], dt.bfloat16, addr_space="Shared")
    m3_own = nc.dram_tensor("m3_own", [ROWS_PER_CORE, FPAD], dt.bfloat16)
    m3_full = nc.dram_tensor("m3_full", [N_ROWS, FPAD], dt.bfloat16, addr_space="Shared")

    # tile index -> AG block, block-local tile offset, end-of-block flag
    k0b = [0]
    for tb in AG_TB:
        k0b.append(k0b[-1] + tb)
    tile_blk = {}
    for b, tb in enumerate(AG_TB):
        for k in range(k0b[b], k0b[b + 1]):
            tile_blk[k] = (b, k - k0b[b], k == k0b[b + 1] - 1)
    gbase = [8 * 128 * k0b[b] for b in range(AG_BLOCKS)]

    AL = mybir.AluOpType
    AF = mybir.ActivationFunctionType
    RG = [list(range(N_CORES))]

    with tile.TileContext(nc) as tc:
        with tc.tile_pool(name="const", bufs=1) as constp, \
             tc.tile_pool(name="big", bufs=1) as bigp, \
             tc.tile_pool(name="xstream", bufs=4) as xp, \
             tc.tile_pool(name="work", bufs=4) as wp, \
             tc.tile_pool(name="gpool", bufs=12) as gp, \
             tc.tile_pool(name="selp", bufs=8) as selp, \
             tc.tile_pool(name="psA", bufs=2, space="PSUM") as psA, \
             tc.tile_pool(name="psB", bufs=2, space="PSUM") as psB, \
             tc.tile_pool(name="psmm", bufs=2, space="PSUM") as psmm:

            # ---- resident constants (M1-critical first; gather tables after M1 code) ----
            w1_t = constp.tile([128, KC1, F1], mybir.dt.bfloat16)
            nc.sync.dma_start(w1_t[:], inp['W1'].rearrange("(kc p) n -> p kc n", p=128))
            cs_t = constp.tile([128, TILES_PER_CORE], mybir.dt.float32)
            nc.sync.dma_start(cs_t[:], inp['csrc_t'][:, :])

            h1t = bigp.tile([128, 2, ROWS_PER_CORE], mybir.dt.bfloat16)  # H1.T

            def sub_ag(m_own, m_full, width, b):
                tb = AG_TB[b]
                nc.gpsimd.collective_compute(
                    "AllGather", AL.bypass, replica_groups=RG,
                    ins=[m_own[b][:, :]],
                    outs=[m_full[gbase[b]:gbase[b] + 8 * tb * 128, :]])

            # ---- phase 1: M1 = (X @ W1) * c_src ----
            blocks = [(i * 128, 128) for i in range(TILES_PER_CORE)]
            for c0, bs in blocks:
                xt = xp.tile([128, KC1, bs], mybir.dt.bfloat16, tag="xt")
                nc.sync.dma_start(
                    xt[:, :, :bs],
                    inp['xT'][:, c0:c0 + bs].rearrange("(kc p) n -> p kc n", p=128))
                for sub in range(bs // 128):
                    t_idx = (c0 + sub * 128) // 128
                    ps = psmm.tile([128, F1], mybir.dt.float32, tag="mm1")
                    for kc in range(KC1):
                        nc.tensor.matmul(ps[:], xt[:, kc, sub * 128:(sub + 1) * 128],
                                         w1_t[:, kc, :], start=(kc == 0), stop=(kc == KC1 - 1))
                    ob = wp.tile([128, F1], mybir.dt.bfloat16, tag="m1o")
                    nc.scalar.activation(ob[:], ps[:], AF.Copy, scale=cs_t[:, t_idx:t_idx + 1])
                    b, koff, blk_end = tile_blk[t_idx]
                    nc.sync.dma_start(m1_own[b][koff * 128:(koff + 1) * 128, :], ob[:])
                    if blk_end:
                        sub_ag(m1_own, m1_full, F1, b)

            # gather-phase constants (loaded behind the M1-critical DMAs)
            w2_t = constp.tile([128, 2, F2], mybir.dt.bfloat16)
            nc.sync.dma_start(w2_t[:], inp['W2'].rearrange("(kc p) n -> p kc n", p=128))
            w3_t = constp.tile([F2, F3], mybir.dt.bfloat16)
            nc.sync.dma_start(w3_t[:], inp['W3'][:, :])
            idx1_t = constp.tile([128, SLOTS // 16], mybir.dt.int16)
            nc.sync.dma_start(idx1_t[:], inp['idx1'][:, :])
            idx23_t = constp.tile([128, SLOTS // 16], mybir.dt.int16)
            nc.sync.dma_start(idx23_t[:], inp['idx23'][:, :])
            dst16_t = constp.tile([128, CHUNKS], mybir.dt.int16)
            nc.sync.dma_start(dst16_t[:], inp['dst16'][:, :])
            iota_t = constp.tile([128, 128], mybir.dt.int16)
            nc.gpsimd.iota(iota_t[:], pattern=[[1, 128]], base=0, channel_multiplier=0)
            cdrep_t = constp.tile([128, ROWS_PER_CORE], mybir.dt.float32)
            nc.sync.dma_start(cdrep_t[:], inp['cdst_rep'][:, :])
            cdpp_t = constp.tile([128, TILES_PER_CORE], mybir.dt.float32)
            nc.sync.dma_start(cdpp_t[:], inp['cdst_pp'][:, :])
            b1_t = constp.tile([128, 2], mybir.dt.float32)
            nc.sync.dma_start(b1_t[:], inp['b1pp'][:, :])
            b2_t = constp.tile([128, F2], mybir.dt.float32)
            nc.sync.dma_start(b2_t[:], inp['b2row'][:, :])
            b3_t = constp.tile([128, F3], mybir.dt.float32)
            nc.sync.dma_start(b3_t[:], inp['b3t'][:, :])

            # ---- agg helper ----
            def agg_layer(m_full, elem, consume_chunk, finish_tile, idx_t):
                cur = {}
                for call in range(N_CALLS):
                    g = gp.tile([128, CALL // 128, elem], mybir.dt.bfloat16, tag=f"g{elem}")
                    nc.gpsimd.dma_gather(
                        g[:], m_full[IDX_OFF:, :],
                        idx_t[:, call * (CALL // 16):(call + 1) * (CALL // 16)],
                        CALL, CALL, elem, queue_num=call % 4)
                    nch = CALL // 128
                    ch0 = call * nch
                    selg = selp.tile([128, nch, 128], mybir.dt.bfloat16, tag="selg", name="selg")
                    nc.vector.tensor_tensor(
                        selg[:],
                        dst16_t[:, ch0:ch0 + nch].unsqueeze(2).broadcast_to([128, nch, 128]),
                        iota_t[:].unsqueeze(1).broadcast_to([128, nch, 128]),
                        AL.is_equal)
                    for j in range(CALL // 128):
                        ch = call * (CALL // 128) + j
                        t_idx = ch // R_CHUNKS
                        first = (ch % R_CHUNKS == 0)
                        last = (ch % R_CHUNKS == R_CHUNKS - 1)
                        sel = selg[:, j, :]
                        consume_chunk(cur, g, j, sel, t_idx, first, last)
                        if last:
                            finish_tile(cur, t_idx)
                            cur.clear()

            # ---- layer 1 aggregation -> H1T ----
            def l1_chunk(cur, g, j, sel, t_idx, first, last):
                if first:
                    cur[0] = psA.tile([128, 2, 128], mybir.dt.float32, tag="aggA", name="psa1")
                for fc in range(2):
                    nc.tensor.matmul(cur[0][:, fc, :], g[:, j, fc * 128:(fc + 1) * 128],
                                     sel, start=first, stop=last)

            # M2 = (H1 @ W2) * c_src, emitted per tile (2-tile lag) so the
            # in-order tensor queue doesn't serialize M2 behind all L1 aggs.
            def emit_m2(t_idx):
                sl = slice(t_idx * 128, (t_idx + 1) * 128)
                ps = psmm.tile([128, F2], mybir.dt.float32, tag="mm1")
                for fc in range(2):
                    nc.tensor.matmul(ps[:], h1t[:, fc, sl], w2_t[:, fc, :],
                                     start=(fc == 0), stop=(fc == 1))
                ob = wp.tile([128, FPAD], mybir.dt.bfloat16, tag="m2o")
                nc.scalar.activation(ob[:, 0:F2], ps[:], AF.Copy, scale=cs_t[:, t_idx:t_idx + 1])
                nc.sync.dma_start(m2_own[t_idx * 128:(t_idx + 1) * 128, :], ob[:])

            def l1_tile(cur, t_idx):
                sl = slice(t_idx * 128, (t_idx + 1) * 128)
                for fc in range(2):
                    nc.vector.tensor_tensor(h1t[:, fc, sl], cur[0][:, fc, :],
                                            cdrep_t[:, sl], AL.mult)
                    nc.scalar.activation(h1t[:, fc, sl], h1t[:, fc, sl],
                                         AF.Relu, bias=b1_t[:, fc:fc + 1])
                if t_idx >= 2:
                    emit_m2(t_idx - 2)
                if t_idx == TILES_PER_CORE - 1:
                    emit_m2(TILES_PER_CORE - 2)
                    emit_m2(TILES_PER_CORE - 1)

            agg_layer(m1_full, F1, l1_chunk, l1_tile, idx1_t)

            nc.gpsimd.collective_compute("AllGather", AL.bypass, replica_groups=RG,
                                         ins=[m2_own[:, :]], outs=[m2_full[:, :]])

            # ---- layer 2 aggregation (node-major) -> H2c rows -> m3_own ----
            # Y3 = A@(H2c@W3) = (A@H2c)@W3, so we AllGather/gather H2c (32-wide)
            # and apply W3 after the layer-3 aggregation.
            def l2_chunk(cur, g, j, sel, t_idx, first, last):
                if first:
                    cur[0] = psA.tile([128, 2, 128], mybir.dt.float32, tag="aggA", name="psa2")
                nc.tensor.matmul(cur[0][:, 0, 0:F2], sel, g[:, j, 0:F2], start=first, stop=last)

            def l2_tile(cur, t_idx):
                tmp = wp.tile([128, F2], mybir.dt.float32, tag="h2f", name="h2f")
                nc.scalar.activation(tmp[:], cur[0][:, 0, 0:F2], AF.Copy,
                                     scale=cdpp_t[:, t_idx:t_idx + 1])
                nc.vector.tensor_tensor(tmp[:], tmp[:], b2_t[:], AL.add)
                ob = wp.tile([128, FPAD], mybir.dt.bfloat16, tag="m3o")
                # relu(x*c_src) == relu(x)*c_src since c_src > 0
                nc.scalar.activation(ob[:, 0:F2], tmp[:], AF.Relu,
                                     scale=cs_t[:, t_idx:t_idx + 1])
                nc.sync.dma_start(m3_own[t_idx * 128:(t_idx + 1) * 128, :], ob[:])

            agg_layer(m2_full, FPAD, l2_chunk, l2_tile, idx23_t)

            nc.gpsimd.collective_compute("AllGather", AL.bypass, replica_groups=RG,
                                         ins=[m3_own[:, :]], outs=[m3_full[:, :]])

            # ---- layer 3 aggregation -> per-tile log_softmax -> out ----
            def l3_chunk(cur, g, j, sel, t_idx, first, last):
                if first:
                    cur[0] = psB.tile([F2, 128], mybir.dt.float32, tag="aggB", name="psa3")
                nc.tensor.matmul(cur[0][:], g[:, j, 0:F2], sel, start=first, stop=last)

            GT = 16  # tiles per batched log_softmax group
            smx = {}

            def l3_tile(cur, t_idx):
                gi = t_idx % GT
                if gi == 0:
                    smx['xg'] = wp.tile([128, GT, F3], mybir.dt.float32, tag="xg", name="xg")
                xg = smx['xg']
                sl = slice(t_idx * 128, (t_idx + 1) * 128)
                hsb = wp.tile([F2, 128], mybir.dt.bfloat16, tag="hsb", name="hsb")
                nc.vector.tensor_tensor(hsb[:], cur[0][:], cdrep_t[0:F2, sl], AL.mult)
                ps3 = psmm.tile([128, F3], mybir.dt.float32, tag="y3", name="y3")
                nc.tensor.matmul(ps3[:], hsb[:], w3_t[:], start=True, stop=True)
                nc.vector.tensor_tensor(xg[:, gi, :], ps3[:], b3_t[:], AL.add)
                if gi == GT - 1 or t_idx == TILES_PER_CORE - 1:
                    n = gi + 1
                    t0 = t_idx - gi
                    ex = wp.tile([128, GT, F3], mybir.dt.float32, tag="exg", name="exg")
                    nc.scalar.activation(ex[:, 0:n, :], xg[:, 0:n, :], AF.Exp)
                    sm = wp.tile([128, GT], mybir.dt.float32, tag="smg", name="smg")
                    nc.vector.tensor_reduce(sm[:, 0:n], ex[:, 0:n, :], mybir.AxisListType.X, AL.add)
                    lns = wp.tile([128, GT], mybir.dt.float32, tag="lng", name="lng")
                    nc.scalar.activation(lns[:, 0:n], sm[:, 0:n], AF.Ln)
                    ox = wp.tile([128, GT, F3], mybir.dt.float32, tag="oxg", name="oxg")
                    nc.vector.tensor_tensor(
                        ox[:, 0:n, :], xg[:, 0:n, :],
                        lns[:, 0:n].unsqueeze(2).broadcast_to([128, n, F3]),
                        AL.subtract)
                    nc.sync.dma_start(
                        out_t[t0 * 128:(t_idx + 1) * 128, :].rearrange(
                            "(i p) f -> p i f", p=128),
                        ox[:, 0:n, :])

            agg_layer(m3_full, FPAD, l3_chunk, l3_tile, idx23_t)

    nc.compile()
    return nc


def _install_profile_shim():
    """Provide the missing antenv.axon_hooks module so trace=True works under axon."""
    try:
        import types
        import antenv
        if 'antenv.axon_hooks' in sys.modules:
            return
        _hook = [None]
        mod = types.ModuleType('antenv.axon_hooks')
        mod.set_axon_ntff_profile_hook = lambda h: _hook.__setitem__(0, h)
        mod.get_axon_ntff_profile_hook = lambda: _hook[0]
        sys.modules['antenv.axon_hooks'] = mod
        antenv.axon_hooks = mod
        from trn_agent_boot.trn_boot import _ntff_profile_via_ctypes
        mod.set_axon_ntff_profile_hook(
            _ntff_profile_via_ctypes('/opt/axon/libaxon_pjrt.so'))
    except Exception:
        pass


_CACHE = {}


def kernel(features, edge_index, W1, b1, W2, b2, W3, b3):
    global last_exec_time_ns
    features = np.asarray(features, dtype=np.float32)
    pre = _preprocess(np.asarray(edge_index))

    if 'nc' not in _CACHE:
        _CACHE['nc'] = _build_nc()
    nc = _CACHE['nc']

    # host-side input prep
    W1p = np.zeros((F_IN_P, F1), dtype=BF16)
    W1p[:F_IN] = np.asarray(W1, dtype=BF16)
    W2b = np.asarray(W2, dtype=BF16)
    W3b = np.asarray(W3, dtype=BF16)
    b1pp = np.asarray(b1, dtype=np.float32).reshape(2, 128).T.copy()
    b2row = np.tile(np.asarray(b2, dtype=np.float32), (128, 1))
    b3t = np.tile(np.asarray(b3, dtype=np.float32), (128, 1))

    # features, permuted and transposed per core: [F_IN_P, 6272] bf16
    feat_b = features.astype(BF16)
    in_maps = []
    for c in range(N_CORES):
        rows = pre['node_of_row'][c * ROWS_PER_CORE:(c + 1) * ROWS_PER_CORE]
        xTc = np.zeros((F_IN_P, ROWS_PER_CORE), dtype=BF16)
        real = rows >= 0
        xTc[:F_IN, real] = feat_b[rows[real]].T
        in_maps.append({
            'xT': xTc, 'W1': W1p, 'W2': W2b, 'W3': W3b,
            'idx1': pre['idx_tile1'][c], 'idx23': pre['idx_tile23'][c],
            'dst16': pre['dst16'][c],
            'cdst_rep': pre['cdst_rep'][c], 'cdst_pp': pre['cdst_pp'][c],
            'csrc_t': pre['csrc_t'][c],
            'b1pp': b1pp, 'b2row': b2row, 'b3t': b3t,
        })

    trace = os.environ.get('BASS_KERNEL_TRACE', '0') == '1'
    if trace:
        _install_profile_shim()
    res = run_bass_kernel_spmd(nc, in_maps, core_ids=list(range(N_CORES)), trace=trace)
    last_exec_time_ns = res.exec_time_ns

    # assemble + inverse permute
    out_rows = np.concatenate([res.results[c]['out'] for c in range(N_CORES)], axis=0)
    out = np.empty((N_NODES, F3), dtype=np.float32)
    real = pre['node_of_row'] >= 0
    out[pre['node_of_row'][real]] = out_rows[real]
    return out



# revision 51
# speedup vs baseline: 1.1503x; 1.1503x over previous
"""Trainium2 Bass kernel for nn_DGL_Net (3-layer GraphConv GNN, 50000 nodes, 800k edges).

Strategy (8 NeuronCores, SPMD):
  - Host: relabel nodes into 392 balanced tiles of 128 nodes (<=2046 in-edges per
    tile), 49 tiles per core. Per layer: local matmul (bf16) -> scale by c_src ->
    AllGather of the per-node activations -> per-edge dma_gather (4 SWDGE queues,
    12-deep pipelining so the drain runs at steady state) -> one-hot (Sel) matmul
    aggregation in PSUM -> scale by c_dst + bias (+relu / log_softmax).
  - AllGather 1 is split into 4 sub-collectives over a block-interleaved global
    row layout so the transfers overlap the tail of the M1 matmuls (L2/L3 use a
    separate core-major layout with their own idx table).
  - Sel one-hots are built on-chip (DVE is_equal of per-slot dst lanes against an
    iota), one [128, 1024] op per gather call; no selc DMA. Padding slots carry
    dst_local=-1 so their Sel column is all-zero.
  - Layer 3 uses Y3 = (A @ H2c) @ W3 (associativity): H2c (32-wide) is
    AllGathered/gathered instead of M3, and W3 is applied per tile after the
    aggregation; there is no separate M3 phase. log_softmax is batched per 8
    tiles to avoid per-tile cross-engine ping-pong.
  - PSUM drains run on the scalar engine (activation Copy/Relu with per-partition
    scale) -- small-free-dim PSUM reads on the DVE are pathologically slow.
  - int16 gather indices: gather base is offset +32768 rows so idx = row-32768
    spans the whole [0, 50176) row space within int16. The last slot of every
    1024-index gather call is a reserved dummy with idx>=0 (defeats the ucode's
    trailing-negative trim). num_idxs per dma_gather call is capped at 1024 by
    the ucode.
"""
import os
import sys

sys.path.insert(0, '/opt/trn_rl_repo')

import numpy as np
import ml_dtypes

import concourse.bass as bass
import concourse.bacc as bacc
import concourse.mybir as mybir
import concourse.tile as tile
from concourse.bass_utils import run_bass_kernel_spmd

BF16 = ml_dtypes.bfloat16

N_NODES = 50000
N_CORES = 8
TILE_N = 128                 # nodes per tile
TILES_PER_CORE = 49
AG_TB = [13, 12, 12, 12]     # tiles per AllGather sub-block (sum = 49)
AG_BLOCKS = len(AG_TB)
N_TILES = N_CORES * TILES_PER_CORE      # 392
ROWS_PER_CORE = TILES_PER_CORE * TILE_N  # 6272
N_ROWS = N_CORES * ROWS_PER_CORE         # 50176
R_CHUNKS = 16                # edge chunks (of 128 slots) per tile
SLOTS_PER_TILE = R_CHUNKS * 128          # 2048
TILE_EDGE_CAP = SLOTS_PER_TILE - 2       # 2046 (2 reserved call-end dummies)
SLOTS = TILES_PER_CORE * SLOTS_PER_TILE  # 100352 per core
CALL = 1024                  # idxs per dma_gather call
N_CALLS = SLOTS // CALL      # 98
CHUNKS = TILES_PER_CORE * R_CHUNKS       # 784 chunks per core
IDX_OFF = 32768              # gather base offset (int16 trick)
F_IN = 1433
F_IN_P = 1536                # padded to 12*128
KC1 = F_IN_P // 128          # 12
F1 = 256
F2 = 32
F3 = 7
FPAD = 128                   # padded row width for M2/M3 gather (256B elems)

last_exec_time_ns = None


def _preprocess(edge_index):
    """Graph preprocessing: normalization constants, node->($core,tile,lane)
    relabeling with balanced per-tile in-degree, per-core edge slot tables."""
    src = np.asarray(edge_index[0], dtype=np.int64)
    dst = np.asarray(edge_index[1], dtype=np.int64)
    n_edges = src.shape[0]

    deg_out = np.bincount(src, minlength=N_NODES).astype(np.float64)
    deg_in = np.bincount(dst, minlength=N_NODES).astype(np.float64)
    c_src = (1.0 / np.sqrt(np.maximum(deg_out, 1.0))).astype(np.float32)
    c_dst = (1.0 / np.sqrt(np.maximum(deg_in, 1.0))).astype(np.float32)

    # --- greedy balanced tile packing by in-degree ---
    import heapq
    order = np.argsort(-deg_in, kind='stable')
    heap = [(0.0, 0, t) for t in range(N_TILES)]  # (load, count, tile)
    heapq.heapify(heap)
    tile_nodes = [[] for _ in range(N_TILES)]
    tile_load = np.zeros(N_TILES)
    deferred = []
    for v in order:
        dv = deg_in[v]
        while True:
            load, cnt, t = heapq.heappop(heap)
            if cnt >= TILE_N:
                continue  # stale/full
            if load + dv > TILE_EDGE_CAP:
                deferred.append((load, cnt, t))
                continue
            break
        tile_nodes[t].append(int(v))
        tile_load[t] = load + dv
        heapq.heappush(heap, (load + dv, cnt + 1, t))
        for item in deferred:
            heapq.heappush(heap, item)
        deferred = []
    assert max(tile_load) <= TILE_EDGE_CAP

    # sort tiles by load desc, group by 8, core c takes c-th of each group
    tsort = np.argsort(-tile_load, kind='stable')
    # tile_of_slot[core][k] = global tile id
    tile_assign = np.empty((N_CORES, TILES_PER_CORE), dtype=np.int64)
    for k in range(TILES_PER_CORE):
        for c in range(N_CORES):
            tile_assign[c, k] = tsort[k * N_CORES + c]

    # local row mapping: lrow = c*ROWS_PER_CORE + k*128 + lane  (per-core I/O order)
    # global row mapping (AllGather result layout, block-interleaved so the
    # AllGather can be split into AG_BLOCKS sub-collectives):
    #   grow = 8*128*sum(AG_TB[:b]) + c*AG_TB[b]*128 + (k-k0[b])*128 + lane
    node_of_row = np.full(N_ROWS, -1, dtype=np.int64)  # -1 = virtual pad node
    grow_of_node = np.full(N_NODES, -1, dtype=np.int64)
    node_c = np.full(N_NODES, -1, dtype=np.int64)
    node_k = np.full(N_NODES, -1, dtype=np.int64)
    node_lane = np.full(N_NODES, -1, dtype=np.int64)
    blk_of_k = np.concatenate([np.full(t, b) for b, t in enumerate(AG_TB)])
    k0_of_blk = np.concatenate([[0], np.cumsum(AG_TB)])[:-1]
    gbase_of_blk = 8 * TILE_N * np.concatenate([[0], np.cumsum(AG_TB)])[:-1]
    for c in range(N_CORES):
        for k in range(TILES_PER_CORE):
            t = tile_assign[c, k]
            nodes = tile_nodes[t]
            base = c * ROWS_PER_CORE + k * TILE_N
            b = blk_of_k[k]
            gbase = gbase_of_blk[b] + c * AG_TB[b] * TILE_N + (k - k0_of_blk[b]) * TILE_N
            for lane, v in enumerate(nodes):
                node_of_row[base + lane] = v
                grow_of_node[v] = gbase + lane
                node_c[v] = c
                node_k[v] = k
                node_lane[v] = lane
    assert (grow_of_node >= 0).all()

    # --- per-core edge slot tables ---
    # L1 gathers read the block-interleaved (split-AG) layout; L2/L3 read the
    # core-major layout of the single AllGathers. Same slots, two idx tables.
    lrow_of_node = node_c * ROWS_PER_CORE + node_k * TILE_N + node_lane
    src_row1 = grow_of_node[src]
    src_row23 = lrow_of_node[src]
    e_core = node_c[dst]
    e_tile = node_k[dst]              # k within core
    e_lane = node_lane[dst]

    idx1_flat = np.zeros((N_CORES, SLOTS), dtype=np.int16)     # pad idx = 0
    idx23_flat = np.zeros((N_CORES, SLOTS), dtype=np.int16)
    dst_flat = np.full((N_CORES, SLOTS), -1, dtype=np.int16)   # pad dst = -1

    # group edges by (core, tile) and assign slot positions
    key = e_core * TILES_PER_CORE + e_tile
    eorder = np.argsort(key, kind='stable')
    key_s = key[eorder]
    # position within group
    grp_start = np.searchsorted(key_s, np.arange(N_CORES * TILES_PER_CORE))
    pos_in_grp = np.arange(n_edges) - grp_start[key_s]
    # slot within tile: skip reserved slots 1023 and 2047
    j = pos_in_grp
    slot_in_tile = j + (j >= 1023).astype(np.int64) # j>=1023 shifts past slot 1023
    assert slot_in_tile.max() < SLOTS_PER_TILE - 1  # never hits 2047
    slots_abs = key_s % TILES_PER_CORE * SLOTS_PER_TILE + slot_in_tile
    cores_s = key_s // TILES_PER_CORE
    idx1_flat[cores_s, slots_abs] = (src_row1[eorder] - IDX_OFF).astype(np.int16)
    idx23_flat[cores_s, slots_abs] = (src_row23[eorder] - IDX_OFF).astype(np.int16)
    dst_flat[cores_s, slots_abs] = e_lane[eorder].astype(np.int16)

    # wrap idx to [128, SLOTS/16] (idx i -> [i%16 replicated, i//16])
    cols = SLOTS // 16
    idx_tile1 = np.zeros((N_CORES, 128, cols), dtype=np.int16)
    idx_tile23 = np.zeros((N_CORES, 128, cols), dtype=np.int16)
    for c in range(N_CORES):
        idx_tile1[c] = np.tile(idx1_flat[c].reshape(cols, 16).T, (8, 1))
        idx_tile23[c] = np.tile(idx23_flat[c].reshape(cols, 16).T, (8, 1))
    # per-slot dst lane, [128 lane, CHUNKS] int16 (-1 pad); Sel one-hots are
    # built on-chip via is_equal against an iota.
    dst16 = dst_flat.reshape(N_CORES, CHUNKS, 128).transpose(0, 2, 1).copy()

    # per-core normalization tables
    cd_row = np.where(node_of_row >= 0, c_dst[np.maximum(node_of_row, 0)], 1.0)
    cs_row = np.where(node_of_row >= 0, c_src[np.maximum(node_of_row, 0)], 1.0)
    cd_core = cd_row.reshape(N_CORES, ROWS_PER_CORE).astype(np.float32)
    cs_core = cs_row.reshape(N_CORES, ROWS_PER_CORE).astype(np.float32)
    cdst_rep = np.repeat(cd_core[:, None, :], 128, axis=1)  # [C,128,6272]
    cdst_pp = cd_core.reshape(N_CORES, TILES_PER_CORE, 128).transpose(0, 2, 1).copy()
    csrc_t = cs_core.reshape(N_CORES, TILES_PER_CORE, 128).transpose(0, 2, 1).copy()

    return dict(node_of_row=node_of_row,
                idx_tile1=idx_tile1, idx_tile23=idx_tile23, dst16=dst16,
                cdst_rep=cdst_rep.astype(np.float32), cdst_pp=cdst_pp,
                csrc_t=csrc_t)


def _build_nc():
    nc = bacc.Bacc("TRN2", target_bir_lowering=False, debug=False,
                   enable_asserts=True, num_devices=N_CORES, num_swdge_queues=4)
    dt = mybir.dt
    inp = {}
    inp['xT'] = nc.dram_tensor("xT", [F_IN_P, ROWS_PER_CORE], dt.bfloat16, kind="ExternalInput")
    inp['W1'] = nc.dram_tensor("W1", [F_IN_P, F1], dt.bfloat16, kind="ExternalInput")
    inp['W2'] = nc.dram_tensor("W2", [F1, F2], dt.bfloat16, kind="ExternalInput")
    inp['W3'] = nc.dram_tensor("W3", [F2, F3], dt.bfloat16, kind="ExternalInput")
    inp['idx1'] = nc.dram_tensor("idx1", [128, SLOTS // 16], dt.int16, kind="ExternalInput")
    inp['idx23'] = nc.dram_tensor("idx23", [128, SLOTS // 16], dt.int16, kind="ExternalInput")
    inp['dst16'] = nc.dram_tensor("dst16", [128, CHUNKS], dt.int16, kind="ExternalInput")
    inp['cdst_rep'] = nc.dram_tensor("cdst_rep", [128, ROWS_PER_CORE], dt.float32, kind="ExternalInput")
    inp['cdst_pp'] = nc.dram_tensor("cdst_pp", [128, TILES_PER_CORE], dt.float32, kind="ExternalInput")
    inp['csrc_t'] = nc.dram_tensor("csrc_t", [128, TILES_PER_CORE], dt.float32, kind="ExternalInput")
    inp['b1pp'] = nc.dram_tensor("b1pp", [128, 2], dt.float32, kind="ExternalInput")
    inp['b2row'] = nc.dram_tensor("b2row", [128, F2], dt.float32, kind="ExternalInput")
    inp['b3t'] = nc.dram_tensor("b3t", [128, F3], dt.float32, kind="ExternalInput")
    out_t = nc.dram_tensor("out", [ROWS_PER_CORE, F3], dt.float32, kind="ExternalOutput")

    m1_own = [nc.dram_tensor(f"m1_own{b}", [tb * 128, F1], dt.bfloat16)
              for b, tb in enumerate(AG_TB)]
    m1_full = nc.dram_tensor("m1_full", [N_ROWS, F1], dt.bfloat16, addr_space="Shared")
    m2_own = nc.dram_tensor("m2_own", [ROWS_PER_CORE, FPAD], dt.bfloat16)
    m2_full = nc.dram_tensor("m2_full", [N_ROWS, FPAD], dt.bfloat16, addr_space="Shared")
    m3_own = nc.dram_tensor("m3_own", [ROWS_PER_CORE, FPAD], dt.bfloat16)
    m3_full = nc.dram_tensor("m3_full", [N_ROWS, FPAD], dt.bfloat16, addr_space="Shared")

    # tile index -> AG block, block-local tile offset, end-of-block flag
    k0b = [0]
    for tb in AG_TB:
        k0b.append(k0b[-1] + tb)
    tile_blk = {}
    for b, tb in enumerate(AG_TB):
        for k in range(k0b[b], k0b[b + 1]):
            tile_blk[k] = (b, k - k0b[b], k == k0b[b + 1] - 1)
    gbase = [8 * 128 * k0b[b] for b in range(AG_BLOCKS)]

    AL = mybir.AluOpType
    AF = mybir.ActivationFunctionType
    RG = [list(range(N_CORES))]

    with tile.TileContext(nc) as tc:
        with tc.tile_pool(name="const", bufs=1) as constp, \
             tc.tile_pool(name="big", bufs=1) as bigp, \
             tc.tile_pool(name="xstream", bufs=2) as xp, \
             tc.tile_pool(name="work", bufs=4) as wp, \
             tc.tile_pool(name="gpool", bufs=12) as gp, \
             tc.tile_pool(name="selp", bufs=6) as selp, \
             tc.tile_pool(name="psA", bufs=2, space="PSUM") as psA, \
             tc.tile_pool(name="psB", bufs=2, space="PSUM") as psB, \
             tc.tile_pool(name="psmm", bufs=2, space="PSUM") as psmm:

            # ---- resident constants (M1-critical first; gather tables after M1 code) ----
            w1_t = constp.tile([128, KC1, F1], mybir.dt.bfloat16)
            nc.sync.dma_start(w1_t[:], inp['W1'].rearrange("(kc p) n -> p kc n", p=128))
            cs_t = constp.tile([128, TILES_PER_CORE], mybir.dt.float32)
            nc.sync.dma_start(cs_t[:], inp['csrc_t'][:, :])

            h1t = bigp.tile([128, 2, ROWS_PER_CORE], mybir.dt.bfloat16)  # H1.T

            def sub_ag(m_own, m_full, width, b):
                tb = AG_TB[b]
                nc.gpsimd.collective_compute(
                    "AllGather", AL.bypass, replica_groups=RG,
                    ins=[m_own[b][:, :]],
                    outs=[m_full[gbase[b]:gbase[b] + 8 * tb * 128, :]])

            # ---- phase 1: M1 = (X @ W1) * c_src ----
            blocks = [(0, 128)] + [(128 + i * 512, 512) for i in range(12)]
            for c0, bs in blocks:
                xt = xp.tile([128, KC1, bs], mybir.dt.bfloat16, tag=f"xt{bs}")
                nc.sync.dma_start(
                    xt[:, :, :bs],
                    inp['xT'][:, c0:c0 + bs].rearrange("(kc p) n -> p kc n", p=128))
                for sub in range(bs // 128):
                    t_idx = (c0 + sub * 128) // 128
                    ps = psmm.tile([128, F1], mybir.dt.float32, tag="mm1")
                    for kc in range(KC1):
                        nc.tensor.matmul(ps[:], xt[:, kc, sub * 128:(sub + 1) * 128],
                                         w1_t[:, kc, :], start=(kc == 0), stop=(kc == KC1 - 1))
                    ob = wp.tile([128, F1], mybir.dt.bfloat16, tag="m1o")
                    nc.scalar.activation(ob[:], ps[:], AF.Copy, scale=cs_t[:, t_idx:t_idx + 1])
                    b, koff, blk_end = tile_blk[t_idx]
                    nc.sync.dma_start(m1_own[b][koff * 128:(koff + 1) * 128, :], ob[:])
                    if blk_end:
                        sub_ag(m1_own, m1_full, F1, b)

            # gather-phase constants (loaded behind the M1-critical DMAs)
            w2_t = constp.tile([128, 2, F2], mybir.dt.bfloat16)
            nc.sync.dma_start(w2_t[:], inp['W2'].rearrange("(kc p) n -> p kc n", p=128))
            w3_t = constp.tile([F2, F3], mybir.dt.bfloat16)
            nc.sync.dma_start(w3_t[:], inp['W3'][:, :])
            idx1_t = constp.tile([128, SLOTS // 16], mybir.dt.int16)
            nc.sync.dma_start(idx1_t[:], inp['idx1'][:, :])
            idx23_t = constp.tile([128, SLOTS // 16], mybir.dt.int16)
            nc.sync.dma_start(idx23_t[:], inp['idx23'][:, :])
            dst16_t = constp.tile([128, CHUNKS], mybir.dt.int16)
            nc.sync.dma_start(dst16_t[:], inp['dst16'][:, :])
            iota_t = constp.tile([128, 128], mybir.dt.int16)
            nc.gpsimd.iota(iota_t[:], pattern=[[1, 128]], base=0, channel_multiplier=0)
            cdrep_t = constp.tile([128, ROWS_PER_CORE], mybir.dt.float32)
            nc.sync.dma_start(cdrep_t[:], inp['cdst_rep'][:, :])
            cdpp_t = constp.tile([128, TILES_PER_CORE], mybir.dt.float32)
            nc.sync.dma_start(cdpp_t[:], inp['cdst_pp'][:, :])
            b1_t = constp.tile([128, 2], mybir.dt.float32)
            nc.sync.dma_start(b1_t[:], inp['b1pp'][:, :])
            b2_t = constp.tile([128, F2], mybir.dt.float32)
            nc.sync.dma_start(b2_t[:], inp['b2row'][:, :])
            b3_t = constp.tile([128, F3], mybir.dt.float32)
            nc.sync.dma_start(b3_t[:], inp['b3t'][:, :])

            # ---- agg helper ----
            def agg_layer(m_full, elem, consume_chunk, finish_tile, idx_t):
                cur = {}
                for call in range(N_CALLS):
                    g = gp.tile([128, CALL // 128, elem], mybir.dt.bfloat16, tag=f"g{elem}")
                    nc.gpsimd.dma_gather(
                        g[:], m_full[IDX_OFF:, :],
                        idx_t[:, call * (CALL // 16):(call + 1) * (CALL // 16)],
                        CALL, CALL, elem, queue_num=call % 4)
                    nch = CALL // 128
                    ch0 = call * nch
                    selg = selp.tile([128, nch, 128], mybir.dt.bfloat16, tag="selg", name="selg")
                    nc.vector.tensor_tensor(
                        selg[:],
                        dst16_t[:, ch0:ch0 + nch].unsqueeze(2).broadcast_to([128, nch, 128]),
                        iota_t[:].unsqueeze(1).broadcast_to([128, nch, 128]),
                        AL.is_equal)
                    for j in range(CALL // 128):
                        ch = call * (CALL // 128) + j
                        t_idx = ch // R_CHUNKS
                        first = (ch % R_CHUNKS == 0)
                        last = (ch % R_CHUNKS == R_CHUNKS - 1)
                        sel = selg[:, j, :]
                        consume_chunk(cur, g, j, sel, t_idx, first, last)
                        if last:
                            finish_tile(cur, t_idx)
                            cur.clear()

            # ---- layer 1 aggregation -> H1T ----
            def l1_chunk(cur, g, j, sel, t_idx, first, last):
                if first:
                    cur[0] = psA.tile([128, 2, 128], mybir.dt.float32, tag="aggA", name="psa1")
                for fc in range(2):
                    nc.tensor.matmul(cur[0][:, fc, :], g[:, j, fc * 128:(fc + 1) * 128],
                                     sel, start=first, stop=last)

            # M2 = (H1 @ W2) * c_src, emitted per tile (2-tile lag) so the
            # in-order tensor queue doesn't serialize M2 behind all L1 aggs.
            def emit_m2(t_idx):
                sl = slice(t_idx * 128, (t_idx + 1) * 128)
                ps = psmm.tile([128, F2], mybir.dt.float32, tag="mm1")
                for fc in range(2):
                    nc.tensor.matmul(ps[:], h1t[:, fc, sl], w2_t[:, fc, :],
                                     start=(fc == 0), stop=(fc == 1))
                ob = wp.tile([128, FPAD], mybir.dt.bfloat16, tag="m2o")
                nc.scalar.activation(ob[:, 0:F2], ps[:], AF.Copy, scale=cs_t[:, t_idx:t_idx + 1])
                nc.sync.dma_start(m2_own[t_idx * 128:(t_idx + 1) * 128, :], ob[:])

            def l1_tile(cur, t_idx):
                sl = slice(t_idx * 128, (t_idx + 1) * 128)
                for fc in range(2):
                    nc.vector.tensor_tensor(h1t[:, fc, sl], cur[0][:, fc, :],
                                            cdrep_t[:, sl], AL.mult)
                    nc.scalar.activation(h1t[:, fc, sl], h1t[:, fc, sl],
                                         AF.Relu, bias=b1_t[:, fc:fc + 1])
                if t_idx >= 2:
                    emit_m2(t_idx - 2)
                if t_idx == TILES_PER_CORE - 1:
                    emit_m2(TILES_PER_CORE - 2)
                    emit_m2(TILES_PER_CORE - 1)

            agg_layer(m1_full, F1, l1_chunk, l1_tile, idx1_t)

            nc.gpsimd.collective_compute("AllGather", AL.bypass, replica_groups=RG,
                                         ins=[m2_own[:, :]], outs=[m2_full[:, :]])

            # ---- layer 2 aggregation (node-major) -> H2c rows -> m3_own ----
            # Y3 = A@(H2c@W3) = (A@H2c)@W3, so we AllGather/gather H2c (32-wide)
            # and apply W3 after the layer-3 aggregation.
            def l2_chunk(cur, g, j, sel, t_idx, first, last):
                if first:
                    cur[0] = psA.tile([128, 2, 128], mybir.dt.float32, tag="aggA", name="psa2")
                nc.tensor.matmul(cur[0][:, 0, 0:F2], sel, g[:, j, 0:F2], start=first, stop=last)

            def l2_tile(cur, t_idx):
                tmp = wp.tile([128, F2], mybir.dt.float32, tag="h2f", name="h2f")
                nc.scalar.activation(tmp[:], cur[0][:, 0, 0:F2], AF.Copy,
                                     scale=cdpp_t[:, t_idx:t_idx + 1])
                nc.vector.tensor_tensor(tmp[:], tmp[:], b2_t[:], AL.add)
                ob = wp.tile([128, FPAD], mybir.dt.bfloat16, tag="m3o")
                # relu(x*c_src) == relu(x)*c_src since c_src > 0
                nc.scalar.activation(ob[:, 0:F2], tmp[:], AF.Relu,
                                     scale=cs_t[:, t_idx:t_idx + 1])
                nc.sync.dma_start(m3_own[t_idx * 128:(t_idx + 1) * 128, :], ob[:])

            agg_layer(m2_full, FPAD, l2_chunk, l2_tile, idx23_t)

            nc.gpsimd.collective_compute("AllGather", AL.bypass, replica_groups=RG,
                                         ins=[m3_own[:, :]], outs=[m3_full[:, :]])

            # ---- layer 3 aggregation -> per-tile log_softmax -> out ----
            def l3_chunk(cur, g, j, sel, t_idx, first, last):
                if first:
                    cur[0] = psB.tile([F2, 128], mybir.dt.float32, tag="aggB", name="psa3")
                nc.tensor.matmul(cur[0][:], g[:, j, 0:F2], sel, start=first, stop=last)

            GT = 16  # tiles per batched log_softmax group
            smx = {}

            def l3_tile(cur, t_idx):
                gi = t_idx % GT
                if gi == 0:
                    smx['xg'] = wp.tile([128, GT, F3], mybir.dt.float32, tag="xg", name="xg")
                xg = smx['xg']
                sl = slice(t_idx * 128, (t_idx + 1) * 128)
                hsb = wp.tile([F2, 128], mybir.dt.bfloat16, tag="hsb", name="hsb")
                nc.vector.tensor_tensor(hsb[:], cur[0][:], cdrep_t[0:F2, sl], AL.mult)
                ps3 = psmm.tile([128, F3], mybir.dt.float32, tag="y3", name="y3")
                nc.tensor.matmul(ps3[:], hsb[:], w3_t[:], start=True, stop=True)
                nc.vector.tensor_tensor(xg[:, gi, :], ps3[:], b3_t[:], AL.add)
                if gi == GT - 1 or t_idx == TILES_PER_CORE - 1:
                    n = gi + 1
                    t0 = t_idx - gi
                    ex = wp.tile([128, GT, F3], mybir.dt.float32, tag="exg", name="exg")
                    nc.scalar.activation(ex[:, 0:n, :], xg[:, 0:n, :], AF.Exp)
                    sm = wp.tile([128, GT], mybir.dt.float32, tag="smg", name="smg")
                    nc.vector.tensor_reduce(sm[:, 0:n], ex[:, 0:n, :], mybir.AxisListType.X, AL.add)
                    lns = wp.tile([128, GT], mybir.dt.float32, tag="lng", name="lng")
                    nc.scalar.activation(lns[:, 0:n], sm[:, 0:n], AF.Ln)
                    ox = wp.tile([128, GT, F3], mybir.dt.float32, tag="oxg", name="oxg")
                    nc.vector.tensor_tensor(
                        ox[:, 0:n, :], xg[:, 0:n, :],
                        lns[:, 0:n].unsqueeze(2).broadcast_to([128, n, F3]),
                        AL.subtract)
                    nc.sync.dma_start(
                        out_t[t0 * 128:(t_idx + 1) * 128, :].rearrange(
                            "(i p) f -> p i f", p=128),
                        ox[:, 0:n, :])

            agg_layer(m3_full, FPAD, l3_chunk, l3_tile, idx23_t)

    nc.compile()
    return nc


def _install_profile_shim():
    """Provide the missing antenv.axon_hooks module so trace=True works under axon."""
    try:
        import types
        import antenv
        if 'antenv.axon_hooks' in sys.modules:
            return
        _hook = [None]
        mod = types.ModuleType('antenv.axon_hooks')
        mod.set_axon_ntff_profile_hook = lambda h: _hook.__setitem__(0, h)
        mod.get_axon_ntff_profile_hook = lambda: _hook[0]
        sys.modules['antenv.axon_hooks'] = mod
        antenv.axon_hooks = mod
        from trn_agent_boot.trn_boot import _ntff_profile_via_ctypes
        mod.set_axon_ntff_profile_hook(
            _ntff_profile_via_ctypes('/opt/axon/libaxon_pjrt.so'))
    except Exception:
        pass


_CACHE = {}


def kernel(features, edge_index, W1, b1, W2, b2, W3, b3):
    global last_exec_time_ns
    features = np.asarray(features, dtype=np.float32)
    pre = _preprocess(np.asarray(edge_index))

    if 'nc' not in _CACHE:
        _CACHE['nc'] = _build_nc()
    nc = _CACHE['nc']

    # host-side input prep
    W1p = np.zeros((F_IN_P, F1), dtype=BF16)
    W1p[:F_IN] = np.asarray(W1, dtype=BF16)
    W2b = np.asarray(W2, dtype=BF16)
    W3b = np.asarray(W3, dtype=BF16)
    b1pp = np.asarray(b1, dtype=np.float32).reshape(2, 128).T.copy()
    b2row = np.tile(np.asarray(b2, dtype=np.float32), (128, 1))
    b3t = np.tile(np.asarray(b3, dtype=np.float32), (128, 1))

    # features, permuted and transposed per core: [F_IN_P, 6272] bf16
    feat_b = features.astype(BF16)
    in_maps = []
    for c in range(N_CORES):
        rows = pre['node_of_row'][c * ROWS_PER_CORE:(c + 1) * ROWS_PER_CORE]
        xTc = np.zeros((F_IN_P, ROWS_PER_CORE), dtype=BF16)
        real = rows >= 0
        xTc[:F_IN, real] = feat_b[rows[real]].T
        in_maps.append({
            'xT': xTc, 'W1': W1p, 'W2': W2b, 'W3': W3b,
            'idx1': pre['idx_tile1'][c], 'idx23': pre['idx_tile23'][c],
            'dst16': pre['dst16'][c],
            'cdst_rep': pre['cdst_rep'][c], 'cdst_pp': pre['cdst_pp'][c],
            'csrc_t': pre['csrc_t'][c],
            'b1pp': b1pp, 'b2row': b2row, 'b3t': b3t,
        })

    trace = os.environ.get('BASS_KERNEL_TRACE', '0') == '1'
    if trace:
        _install_profile_shim()
    res = run_bass_kernel_spmd(nc, in_maps, core_ids=list(range(N_CORES)), trace=trace)
    last_exec_time_ns = res.exec_time_ns

    # assemble + inverse permute
    out_rows = np.concatenate([res.results[c]['out'] for c in range(N_CORES)], axis=0)
    out = np.empty((N_NODES, F3), dtype=np.float32)
    real = pre['node_of_row'] >= 0
    out[pre['node_of_row'][real]] = out_rows[real]
    return out

